# revision 28
# baseline (speedup 1.0000x reference)
"""Multi-head attention (B=4, S=2048, D=1024, H=16) on 8 trn2 NeuronCores.

Sharding (load-balanced tensor/data parallel):
  Batches are sorted by valid_len and paired heaviest-with-lightest. Core c
  handles batch pair p = c//4 (two batches, "slots" 0/1) and head-quarter
  g = c%4 (4 heads, 256 of the 1024 embedding dims). W_Q/W_K/W_V are
  column-sharded, W_O row-sharded; each core emits one partial transposed
  output per slot batch and the host sums the four partials per batch.

valid_lens specialization: the program is compiled for the per-slot maximum
k-block count NKB_j = max over pairs of ceil(valid_len/128). k-blocks past
NKB_j are fully masked for every batch in that slot; the reference maps
masked scores to 1e-9, so each masked key contributes exp(~0) = 1.0 weight.
Those blocks are skipped in the scores/exp/AV pipeline and replaced by a
per-head "masked sum": msum[e] = sum of v over masked keys (a ones-vector
matmul, with the ones-column of v also counting the masked keys for the
softmax denominator). The normalize step computes
  ctx = (ctx_valid + msum) / (denom_valid + count).

Per-core layout (see kernel_v1.py for the unspecialized variant):
  - Host passes X^T so contraction dims sit on partitions.
  - q/k produced transposed [e, s]; v natural [s, e] with a ones column per
    head (65-wide head stride).
  - scoresT[k, q] = k_blk @ q^T: the exp'd tile is directly the stationary
    operand of the AV matmul (no transposes anywhere).
  - Masking inside the Exp activation via per-partition {0,1} scale
    (exp(0) = 1.0 = exp(1e-9) in fp32).
  - No max subtraction: scores ~ N(0,1), fp32 exp cannot overflow.
"""

import math

import numpy as np
import ml_dtypes

import concourse.bass as bass
import concourse.tile as tile
from concourse import mybir
from concourse.bass_utils import run_bass_kernel_spmd

# The walrus build in this container rejects instructions carrying more than
# one semaphore wait ("Too many sync wait commands"), while Tile's scheduler
# freely attaches several. Post-pass: hoist extra waits onto nop instructions
# injected just before the offender on the same engine queue (engines execute
# their queue in order, so the semantics are identical).
def _split_multi_waits(nc, limit=1):
    fn = nc.m.functions[0]
    for b in fn.blocks:
        new = []
        changed = False
        for inst in b.instructions:
            si = inst.sync_info
            waits = list(si.on_wait) if si is not None else []
            if len(waits) > limit:
                for w in waits[:-limit]:
                    nop = mybir.InstNoOp(
                        name=nc.get_next_instruction_name(), ins=[], outs=[]
                    )
                    nop.engine = inst.engine
                    nop.sync_info = mybir.SyncInfo(on_wait=[w], on_update=[])
                    nc.register_instruction(nop)
                    new.append(nop)
                inst.sync_info = mybir.SyncInfo(
                    on_wait=waits[-limit:], on_update=si.on_update
                )
                changed = True
            new.append(inst)
        if changed:
            b.instructions = new


B, S, D, H = 4, 2048, 1024, 16
DH = D // H            # 64 head dim
HL = H // 4            # 4 heads per core
E = HL * DH            # 256 per-core head width
P = 128
SC = 512               # psum bank width in f32 (max matmul N)
NCH = S // SC          # 4 projection chunks
SCE = 1024             # attention q-chunk (ACT overhead amortization)
NCHE = S // SCE        # 2 attention chunks
NSUB = SCE // SC       # matmul sub-chunks per attention chunk
KB = S // P            # 16 k-blocks
DT = D // P            # 8 contraction tiles
ET = E // P            # 2 e-tiles
OB = D // P            # 8 output-row blocks
VW = DH + 1            # 65: head width in v (with ones column)

BF16 = mybir.dt.bfloat16
F32 = mybir.dt.float32
npbf16 = ml_dtypes.bfloat16


def build_nc(nkb):
    """nkb = (NKB_0, NKB_1): per-slot k-block counts (1..16)."""
    nc = bass.Bass()
    x_d = {}
    sel_d = {}
    out_d = {}
    for sl in range(2):
        x_d[sl] = [
            nc.dram_tensor(f"x{nm}{sl}", [D, S], BF16, kind="ExternalInput")
            for nm in ("q", "k", "v")
        ]
        sel_d[sl] = nc.dram_tensor(f"sel{sl}", [P, KB], F32, kind="ExternalInput")
        out_d[sl] = nc.dram_tensor(f"outT{sl}", [D, S], F32, kind="ExternalOutput")
    wq_d = nc.dram_tensor("wq", [D, E], BF16, kind="ExternalInput")
    wk_d = nc.dram_tensor("wk", [D, E], BF16, kind="ExternalInput")
    wv_d = nc.dram_tensor("wv", [D, E], BF16, kind="ExternalInput")
    wo_d = nc.dram_tensor("wo", [E, D], BF16, kind="ExternalInput")

    nchk = [min(NCH, (nkb[sl] * P + SC - 1) // SC) for sl in range(2)]

    with tile.TileContext(nc) as tc:
        with (
            tc.tile_pool(name="wpool", bufs=1) as wpool,
            tc.tile_pool(name="xpool", bufs=12) as xpool,
            tc.tile_pool(name="qkv", bufs=1) as qkv,
            tc.tile_pool(name="expp", bufs=6) as expp,
            tc.tile_pool(name="ctxp", bufs=8) as ctxp,
            tc.tile_pool(name="msump", bufs=8) as msump,
            tc.tile_pool(name="recp", bufs=6) as recp,
            tc.tile_pool(name="bcp", bufs=6) as bcp,
            tc.tile_pool(name="outp", bufs=4) as outp,
            tc.tile_pool(name="ps_sc", bufs=2, space="PSUM") as ps_sc,
            tc.tile_pool(name="ps_ctx", bufs=4, space="PSUM") as ps_ctx,
        ):
            wq_sb = wpool.tile([P, DT, E], BF16)
            wk_sb = wpool.tile([P, DT, E], BF16)
            wv_sb = wpool.tile([P, DT, E], BF16)
            wo_sb = wpool.tile([P, ET, D], BF16)
            ones_sb = wpool.tile([P, 1], BF16)
            nc.sync.dma_start(wq_sb, wq_d.rearrange("(t p) n -> p t n", p=P))
            nc.sync.dma_start(wk_sb, wk_d.rearrange("(t p) n -> p t n", p=P))
            nc.sync.dma_start(wv_sb, wv_d.rearrange("(t p) n -> p t n", p=P))
            nc.sync.dma_start(wo_sb, wo_d.rearrange("(t p) n -> p t n", p=P))
            nc.vector.memset(ones_sb, 1.0)

            sel_sb = {}
            qT_sb = {}
            kT_sb = {}
            v_sb = {}
            for sl in range(2):
                ssb = wpool.tile([P, KB], F32, name=f"sel_sb{sl}")
                sel_sb[sl] = ssb
                nc.sync.dma_start(ssb, sel_d[sl][:, :])
                qT_sb[sl] = qkv.tile([P, ET, S], BF16, name=f"qT{sl}")
                kT_sb[sl] = qkv.tile([P, ET, S], BF16, name=f"kT{sl}")
                v_sb[sl] = qkv.tile([P, KB, HL * VW], BF16, name=f"v{sl}")
                nc.vector.memset(
                    v_sb[sl].rearrange("p t (h c) -> p t h c", c=VW)[
                        :, :, :, DH : DH + 1
                    ],
                    1.0,
                )

            def load_x(x_dram, width=S):
                xt = []
                for dt in range(DT):
                    xtile = xpool.tile([P, S], BF16, tag="xt", name="xt")
                    eng = nc.sync if dt % 2 == 0 else nc.gpsimd
                    eng.dma_start(
                        xtile[:, 0:width], x_dram[dt * P : (dt + 1) * P, 0:width]
                    )
                    xt.append(xtile)
                return xt

            # Projection work is emitted as closures ("fillers") so it can be
            # interleaved between attention matmuls: the in-order PE queue
            # stalls at each AV matmul until its exp is done, and those
            # micro-stalls keep the HAM clock-gate at 1.2 GHz. A filler group
            # of always-ready matmuls emitted between scores and AV keeps the
            # PE dense (warm) while ACT computes the exp.
            def proj_groups(xt, w_sb, out_sb, nchunks=NCH):
                gs = []
                for et in range(ET):
                    for sc_i in range(nchunks):
                        def g(et=et, sc_i=sc_i, xt=xt, w_sb=w_sb, out_sb=out_sb):
                            ps = ps_sc.tile([P, SC], F32, tag="sc", name="ps")
                            for dt in range(DT):
                                nc.tensor.matmul(
                                    ps,
                                    lhsT=w_sb[:, dt, et * P : (et + 1) * P],
                                    rhs=xt[dt][:, sc_i * SC : (sc_i + 1) * SC],
                                    start=(dt == 0),
                                    stop=(dt == DT - 1),
                                )
                            nc.vector.tensor_copy(
                                out_sb[:, et, sc_i * SC : (sc_i + 1) * SC], ps
                            )
                        gs.append(g)
                return gs

            def vproj_groups(xt, sl):
                # v natural: v[s, e] ; lhsT = X.T[d, s-block], rhs = W[d, e]
                gs = []
                for st in range(KB):
                    def g(st=st, xt=xt, sl=sl):
                        ps = ps_sc.tile([P, E], F32, tag="sc", name="ps")
                        for dt in range(DT):
                            nc.tensor.matmul(
                                ps,
                                lhsT=xt[dt][:, st * P : (st + 1) * P],
                                rhs=wv_sb[:, dt, :],
                                start=(dt == 0),
                                stop=(dt == DT - 1),
                            )
                        nc.vector.tensor_copy(
                            v_sb[sl][:, st].rearrange("p (h c) -> p h c", c=VW)[
                                :, :, 0:DH
                            ],
                            ps.rearrange("p (h c) -> p h c", c=DH),
                        )
                    gs.append(g)
                return gs

            # masked sums: msum[e] = sum of v over fully-masked k-blocks,
            # msum[64] = count of masked keys (via the ones column).
            msum_sb = {}

            def msum_group(sl, h):
                def g(sl=sl, h=h):
                    ms = msump.tile([VW, 1], F32, name=f"ms{sl}_{h}")
                    msum_sb[(sl, h)] = ms
                    if nkb[sl] < KB:
                        mp = ps_ctx.tile([VW, 1], F32, tag="ctx", name="mp")
                        for i, kb in enumerate(range(nkb[sl], KB)):
                            nc.tensor.matmul(
                                mp,
                                lhsT=v_sb[sl][:, kb, h * VW : (h + 1) * VW],
                                rhs=ones_sb,
                                start=(i == 0),
                                stop=(kb == KB - 1),
                            )
                        nc.vector.tensor_copy(ms, mp)
                    else:
                        nc.vector.memset(ms, 0.0)
                return g

            def oproj_groups(ch, ctx_tiles):
                gs = []
                for sl in range(2):
                    for ob in range(OB):
                        for j in range(NSUB):
                            def g(sl=sl, ob=ob, j=j, ch=ch, ctx_tiles=ctx_tiles):
                                ps = ps_sc.tile([P, SC], F32, tag="sc", name="ps")
                                for et in range(ET):
                                    nc.tensor.matmul(
                                        ps,
                                        lhsT=wo_sb[:, et, ob * P : (ob + 1) * P],
                                        rhs=ctx_tiles[(sl, et)][
                                            :, j * SC : (j + 1) * SC
                                        ],
                                        start=(et == 0),
                                        stop=(et == ET - 1),
                                    )
                                ost = outp.tile([P, SC], F32, tag="ost", name="ost")
                                nc.vector.tensor_copy(ost, ps)
                                nc.sync.dma_start(
                                    out_d[sl][
                                        ob * P : (ob + 1) * P,
                                        ch * SCE + j * SC : ch * SCE + (j + 1) * SC,
                                    ],
                                    ost,
                                )
                            gs.append(g)
                return gs

            def attn_unit(ch, sl, h, ctx_tiles, fillers, pops=1):
                et, ro = h // 2, (h % 2) * DH
                ctx_ps = [
                    ps_ctx.tile([VW, SC], F32, tag="ctx", name="ctx_ps")
                    for _ in range(NSUB)
                ]
                for kb in range(nkb[sl]):
                    sc_ps = ps_sc.tile([P, SCE], F32, tag="sc", name="sc_ps")
                    for j in range(NSUB):
                        nc.tensor.matmul(
                            sc_ps[:, j * SC : (j + 1) * SC],
                            lhsT=kT_sb[sl][ro : ro + DH, et, kb * P : (kb + 1) * P],
                            rhs=qT_sb[sl][
                                ro : ro + DH,
                                et,
                                ch * SCE + j * SC : ch * SCE + (j + 1) * SC,
                            ],
                            start=True,
                            stop=True,
                        )
                    ex = expp.tile([P, SCE], BF16, tag="ex", name="ex")
                    nc.scalar.activation(
                        ex,
                        sc_ps,
                        mybir.ActivationFunctionType.Exp,
                        scale=sel_sb[sl][:, kb : kb + 1],
                    )
                    for _ in range(pops):
                        if fillers:
                            fillers.pop(0)()
                    for j in range(NSUB):
                        nc.tensor.matmul(
                            ctx_ps[j],
                            lhsT=v_sb[sl][:, kb, h * VW : (h + 1) * VW],
                            rhs=ex[:, j * SC : (j + 1) * SC],
                            start=(kb == 0),
                            stop=(kb == nkb[sl] - 1),
                        )
                ms = msum_sb[(sl, h)]
                key = (sl, et)
                if key not in ctx_tiles:
                    ctx_tiles[key] = ctxp.tile([P, SCE], BF16, tag="ct", name="ct")
                for j in range(NSUB):
                    den = recp.tile([1, SC], F32, tag="den", name="den")
                    nc.vector.tensor_scalar_add(
                        den, ctx_ps[j][DH : DH + 1, :], ms[DH : DH + 1, 0:1]
                    )
                    rec = recp.tile([1, SC], F32, tag="rec", name="rec")
                    nc.vector.reciprocal(rec, den)
                    bc_sb = bcp.tile([DH, SC], F32, tag="bc", name="bc_sb")
                    rec_bcast = bass.AP(
                        tensor=rec.tensor,
                        offset=rec.offset,
                        ap=[list(rec.ap[0]), [0, DH], list(rec.ap[1])],
                    )
                    nc.gpsimd.dma_start(bc_sb, rec_bcast)
                    nc.vector.scalar_tensor_tensor(
                        ctx_tiles[key][ro : ro + DH, j * SC : (j + 1) * SC],
                        ctx_ps[j][0:DH, :],
                        ms[0:DH, 0:1],
                        bc_sb,
                        mybir.AluOpType.add,
                        mybir.AluOpType.mult,
                    )

            # Phase A: slot 0 projections inline (PE-dense on their own).
            for g in proj_groups(load_x(x_d[0][0]), wq_sb, qT_sb[0]):
                g()
            for g in proj_groups(
                load_x(x_d[0][1], width=nchk[0] * SC), wk_sb, kT_sb[0],
                nchunks=nchk[0],
            ):
                g()
            for g in vproj_groups(load_x(x_d[0][2]), 0):
                g()
            for h in range(HL):
                msum_group(0, h)()

            # Phase B: slot 1 loads queued; its projections become fillers.
            fillers = []
            fillers += proj_groups(load_x(x_d[1][0]), wq_sb, qT_sb[1])
            fillers += proj_groups(
                load_x(x_d[1][1], width=nchk[1] * SC), wk_sb, kT_sb[1],
                nchunks=nchk[1],
            )
            fillers += vproj_groups(load_x(x_d[1][2]), 1)
            for h in range(HL):
                fillers.append(msum_group(1, h))

            # chunk 0: slot-0 units absorb slot-1 projection fillers.
            ctx_tiles0 = {}
            for h in range(HL):
                attn_unit(0, 0, h, ctx_tiles0, fillers)
            while fillers:
                fillers.pop(0)()
            for h in range(HL):
                attn_unit(0, 1, h, ctx_tiles0, fillers)

            # chunk 1: units absorb chunk-0's output projection as fillers.
            fillers = oproj_groups(0, ctx_tiles0)
            ctx_tiles1 = {}
            for hu in range(2 * HL):
                sl, h = hu % 2, hu // 2
                attn_unit(1, sl, h, ctx_tiles1, fillers, pops=2)
            while fillers:
                fillers.pop(0)()
            for g in oproj_groups(1, ctx_tiles1):
                g()

    _split_multi_waits(nc)
    return nc


def plan_shards(valid_lens):
    """Sort batches by valid_len desc, pair heaviest+lightest.

    Returns (pairs, nkb): pairs[p] = (batch_slot0, batch_slot1); nkb[j] is
    the compile-time k-block count for slot j (max over the two pairs)."""
    order = sorted(range(B), key=lambda b: -int(valid_lens[b]))
    pairs = [(order[0], order[3]), (order[1], order[2])]
    nkb = []
    for j in range(2):
        m = max(int(valid_lens[pairs[p][j]]) for p in range(2))
        nkb.append(min(KB, max(1, math.ceil(m / P))))
    return pairs, tuple(nkb)


def make_in_maps(Q, K, V, valid_lens, Wq, Wk, Wv, Wo):
    pairs, nkb = plan_shards(valid_lens)
    xT = {}
    sel = {}
    for b in range(B):
        xT[b] = (
            np.ascontiguousarray(Q[b].T).astype(npbf16),
            np.ascontiguousarray(K[b].T).astype(npbf16),
            np.ascontiguousarray(V[b].T).astype(npbf16),
        )
        s = (np.arange(S) < int(valid_lens[b])).astype(np.float32)
        sel[b] = np.ascontiguousarray(s.reshape(KB, P).T)
    wshard = {}
    for g in range(4):
        cols = slice(g * E, (g + 1) * E)
        wshard[g] = (
            (Wq[:, cols] / 8.0).astype(npbf16),
            Wk[:, cols].astype(npbf16),
            Wv[:, cols].astype(npbf16),
            np.ascontiguousarray(Wo[cols, :]).astype(npbf16),
        )
    in_maps = []
    for c in range(8):
        p, g = c // 4, c % 4
        wq, wk, wv, wo = wshard[g]
        m = {"wq": wq, "wk": wk, "wv": wv, "wo": wo}
        for sl in range(2):
            b = pairs[p][sl]
            m[f"xq{sl}"], m[f"xk{sl}"], m[f"xv{sl}"] = xT[b]
            m[f"sel{sl}"] = sel[b]
        in_maps.append(m)
    return in_maps


_NC_CACHE = {}


def kernel(Q, K, V, valid_lens, Wq, Wk, Wv, Wo):
    Q = np.asarray(Q, dtype=np.float32)
    K = np.asarray(K, dtype=np.float32)
    V = np.asarray(V, dtype=np.float32)
    Wq = np.asarray(Wq, dtype=np.float32)
    Wk = np.asarray(Wk, dtype=np.float32)
    Wv = np.asarray(Wv, dtype=np.float32)
    Wo = np.asarray(Wo, dtype=np.float32)
    valid_lens = np.asarray(valid_lens)

    pairs, nkb = plan_shards(valid_lens)
    in_maps = make_in_maps(Q, K, V, valid_lens, Wq, Wk, Wv, Wo)
    if nkb not in _NC_CACHE:
        _NC_CACHE[nkb] = build_nc(nkb)
    nc = _NC_CACHE[nkb]
    res = run_bass_kernel_spmd(nc, in_maps, core_ids=list(range(8)))
    out = np.empty((B, S, D), np.float32)
    for p in range(2):
        for sl in range(2):
            b = pairs[p][sl]
            acc = res.results[4 * p][f"outT{sl}"].copy()
            for g in range(1, 4):
                acc += res.results[4 * p + g][f"outT{sl}"]
            out[b] = acc.T
    return out
